# revision 17
# baseline (speedup 1.0000x reference)
"""Trainium2 Bass kernel for per-sample dynamic depthwise 3x3 conv + 1x1 conv + BN + ReLU.

Computation (per sample b):
    xn[c, p]  = sum_{dy,dx} k[b, c, dy, dx] * x[b, c, p + shift(dy,dx)]   (depthwise)
    y[o, p]   = sum_c pw[o, c] * xn[c, p]                                  (1x1 conv)
    out       = relu(y * inv[o] + beta_eff[o])                             (BN + ReLU)

Kernel strategy: fold the depthwise conv into the pointwise matmul.  For a
tap t handled on the TensorEngine, the weight matrix
W_t[c, o] = pw[o, c] * inv[o] * k[b, c, t] is built with one per-partition
tensor_scalar multiply ([128, 128], cheap) and the PE accumulates
W_t.T @ x_shift_t into PSUM; the shifted inputs are just access patterns
into a zero-padded fp16 image in SBUF.  To balance engines, N_DVE of the 9
taps are instead computed as a partial depthwise sum xn_B on the
VectorEngine (per-partition scalar multiply + add), folded in via one extra
accumulating matmul with the unscaled weights.  ScalarE applies
relu(psum + beta_eff) directly on PSUM.

Sharding: data-parallel over batch B=32 across 8 cores (4 samples per core);
pw/BN parameters replicated.
"""

import os

import numpy as np

B, C, H, W = 32, 128, 96, 96
KH, KW = 3, 3
BN_EPS = 1e-5
HW = H * W
NCORES = 8
BPC = B // NCORES  # samples per core

# Padded image layout in SBUF: rows 0 and PH-1 are zero, cols 0,1 and
# PW-2, PW-1 are zero.  Pixel (h, w) lives at [h+1, w+2].  Tap (dy, dx)
# for output pixel (h, w) reads [h+dy, w+dx+1].  Left pad of 2 keeps the
# interior start 4B-aligned for the DVE fp32->fp16 cast and the DVE taps.
PH, PW = H + 2, W + 4

CH = 4  # image rows per matmul chunk
NCHUNK = H // CH  # 24 chunks per sample
NFREE = CH * W  # 384 <= 512 fp32 PSUM bank limit
HALF = HW // 2  # output staging granularity

# Taps computed on the VectorEngine: the middle column (dx==1) reads the
# padded image at 4B-aligned offsets, keeping DVE perf modes available.
N_DVE = 3
DVE_TAPS = [(0, 1), (1, 1), (2, 1)][:N_DVE]
PE_TAPS = [
    (dy, dx) for dy in range(3) for dx in range(3) if (dy, dx) not in DVE_TAPS
]

_compiled = None

# production build configuration (see _build for knob meanings).
# gps_copies is OFF: GpSimd TensorCopy measured ~100us/pass slower on real
# silicon than the cost model's efficiency guess.
BEST_KW = dict(
    act_tap=True,
    f16_in=True,
    xnb_parts=4,
    f16_out=True,
    bn_pair=True,
    psum_bufs=8,
    s0_chunks=6,
)


def _build(repeat=1, loop_iters=None, n_dve=N_DVE, half_tap=False, gps_tap=False, psum_bufs=6, s0_full_pe=False, split_xnb=False, extra_rows=None, act_tap=False, cast_split=False, deep_xnb=False, skew=False, bn_pair=False, f16_in=False, dma_direct=False, f16_out=False, xnb_parts=None, gps_copies=False, gps_add_rows=0, s0_chunks=0, f16_w=False, hoist=False, act2_rows=0, w_dma=False, s1_chunks=0, act_copies=0, extra2_rows=0, bf16=False, bn_gps=0, extra2_act=False, w_act=False):
    """Build and compile the per-core Bass program (identical on all cores).

    repeat/loop_iters multiply the body inside the NEFF — used only by the
    timing harness (wall-clock slope isolates per-iteration HW time from
    dispatch overhead).
    """
    from contextlib import ExitStack

    from concourse import bacc, mybir, tile

    f32 = mybir.dt.float32
    # "f16" below is the generic 2-byte compute dtype; bf16 swaps it for
    # bfloat16 (PE streams bf16 slightly faster than fp16 on silicon).
    f16 = mybir.dt.bfloat16 if bf16 else mybir.dt.float16
    out16 = mybir.dt.float16  # output staging stays IEEE fp16

    dve_taps = [(0, 1), (1, 1), (2, 1)][:n_dve]
    # second off-PE tap: rows [0, extra2_rows) of tap (1, 2) are computed as
    # an ACT product + DVE merge into xn_b; the PE covers the remaining rows.
    # Must be a multiple of CH so chunks are either fully off-PE or fully on.
    EXTRA2 = (1, 2)
    assert extra2_rows % CH == 0
    # extra tap whose product is partly computed off-PE: DVE covers rows
    # [0, extra_rows) (PE covers the rest); gps_tap -> full rows on GpSimd.
    if half_tap and extra_rows is None:
        extra_rows = 48
    if act_tap:
        extra_rows = H  # whole tap off the PE; product computed on ScalarE
    extra_tap = (1, 0) if (extra_rows or gps_tap) else None
    off_pe_extra = gps_tap or act_tap  # extra tap never appears in pe_taps
    pe_taps = [
        (dy, dx)
        for dy in range(3)
        for dx in range(3)
        if (dy, dx) not in dve_taps
        and (not off_pe_extra or (dy, dx) != extra_tap)
    ]

    nc = bacc.Bacc(
        "TRN2", target_bir_lowering=False, debug=False, enable_asserts=False
    )
    x_d = nc.dram_tensor(
        "x", [BPC, C, HW], f16 if f16_in else f32, kind="ExternalInput"
    ).ap()
    k_d = nc.dram_tensor("k", [BPC, C, 9], f32, kind="ExternalInput").ap()
    w_d = nc.dram_tensor("w", [C, C], f16 if f16_w else f32, kind="ExternalInput").ap()
    wall_d = (
        nc.dram_tensor("wall", [BPC, C, 9 * C], f16, kind="ExternalInput").ap()
        if w_dma
        else None
    )
    beta_d = nc.dram_tensor("beta", [C, 1], f32, kind="ExternalInput").ap()
    out_dt = out16 if f16_out else f32
    out_d = nc.dram_tensor("out", [BPC, C, HW], out_dt, kind="ExternalOutput").ap()

    with tile.TileContext(nc) as tc, ExitStack() as ctx:
        consts = ctx.enter_context(tc.tile_pool(name="consts", bufs=1))
        xraw_pool = ctx.enter_context(tc.tile_pool(name="xraw", bufs=4))
        xpad_pool = ctx.enter_context(
            tc.tile_pool(name="xpad", bufs=2 if deep_xnb else 3)
        )
        xnb_pool = ctx.enter_context(
            tc.tile_pool(name="xnb", bufs=3 if (deep_xnb or not act_tap) else 2)
        )
        tmp_pool = ctx.enter_context(tc.tile_pool(name="tmp", bufs=1))
        tmpa_pool = (
            ctx.enter_context(tc.tile_pool(name="tmpa", bufs=1)) if act_tap else None
        )
        tmpb_pool = (
            ctx.enter_context(tc.tile_pool(name="tmpb", bufs=1))
            if extra2_rows
            else None
        )
        wpool = ctx.enter_context(tc.tile_pool(name="wt", bufs=3 if hoist else 2))
        kpool = ctx.enter_context(tc.tile_pool(name="kt", bufs=3 if hoist else 2))
        opool = ctx.enter_context(tc.tile_pool(name="ot", bufs=3))
        pspool = ctx.enter_context(
            tc.tile_pool(
                name="ps",
                bufs=min(4, max(2, psum_bufs // 2)) if bn_pair else psum_bufs,
                space="PSUM",
            )
        )

        # consts ride the Activation engine's DGE queue so they don't take
        # the SP queue's first (serialized ~650ns each) issue slots away
        # from the startup-critical x/k DMAs.
        w_sb = consts.tile([C, C], f16 if f16_w else f32)
        nc.scalar.dma_start(w_sb[:], w_d)
        beta_sb = consts.tile([C, 1], f32)
        nc.scalar.dma_start(beta_sb[:], beta_d)
        if f16_w:
            w16 = w_sb  # already fp16 from the host; no cast needed
        else:
            w16 = consts.tile([C, C], f16)
            nc.vector.tensor_copy(w16[:], w_sb[:])

        relu = mybir.ActivationFunctionType.Relu

        if loop_iters is not None:
            ctx.enter_context(tc.For_i(0, loop_iters, 1))

        QROWS = H // 4

        def _placement(bi, hh, x_pad, x_raw, q0_splits):
            r0 = 1 + hh * QROWS
            if f16_in and gps_copies and not (bi == 0 and hh == 0):
                # placement copies on the (otherwise idle) GpSimd engine
                # to keep DVE free for the xn_b partial sums.  The very
                # first quarter stays on DVE (idle at startup) so PE's
                # first matmul isn't behind the slower GpSimd copy.
                nc.gpsimd.tensor_copy(
                    x_pad[:, r0 : r0 + QROWS, 2 : W + 2], x_raw[:]
                )
            elif f16_in and act_copies and hh >= 4 - act_copies:
                # rebalance: the last act_copies quarters are placed by the
                # ScalarEngine (headroom) instead of DVE, freeing DVE for
                # the xn_b partial sums at sample boundaries.
                nc.scalar.activation(
                    x_pad[:, r0 : r0 + QROWS, 2 : W + 2],
                    x_raw[:],
                    mybir.ActivationFunctionType.Copy,
                    bias=0.0,
                    scale=1.0,
                )
            elif f16_in:
                # fp16->fp16 placement copy into the padded image; DVE
                # hits 4x mode on these.  For sample 0's first quarter,
                # copy in the same splits the DMA used so chunk 0
                # unblocks as soon as 8 rows land.
                if bi == 0 and hh == 0 and q0_splits is not None:
                    rprev = 0
                    for rsp in q0_splits:
                        nc.vector.tensor_copy(
                            x_pad[:, r0 + rprev : r0 + rsp, 2 : W + 2],
                            x_raw[:, rprev:rsp, :],
                        )
                        rprev = rsp
                else:
                    nc.vector.tensor_copy(
                        x_pad[:, r0 : r0 + QROWS, 2 : W + 2], x_raw[:]
                    )
            elif cast_split and hh % 2 == 1:
                nc.vector.tensor_copy(
                    x_pad[:, r0 : r0 + QROWS, 2 : W + 2], x_raw[:]
                )
            else:
                nc.scalar.activation(
                    x_pad[:, r0 : r0 + QROWS, 2 : W + 2],
                    x_raw[:],
                    mybir.ActivationFunctionType.Copy,
                    bias=0.0,
                    scale=1.0,
                )

        def emit_prep_w(bi, b):
            """Early phase: k/q0 DMAs, w_all weight builds, padded-image
            alloc+memsets, quarter-0 placement.  Emitted one sample ahead
            (hoisted) so the PE never waits on next-sample weights."""
            # sample 0 runs all taps on the PE so it has no DVE dependency:
            # the PE starts immediately while DVE/ACT prepare later samples.
            if s0_full_pe and bi == 0:
                s_dve_taps, s_extra, s_xrows = [], None, 0
                s_pe_taps = [(dy, dx) for dy in range(3) for dx in range(3)]
            else:
                s_dve_taps, s_extra, s_xrows = dve_taps, extra_tap, extra_rows or 0
                s_pe_taps = pe_taps

            # sample-0-lite: the first s0_chunks chunks of sample 0 run all
            # 9 taps directly on the PE (no xn_b dependency), so the PE
            # starts as soon as quarter 0 lands while DVE builds the rest.
            # s1_chunks does the same for sample 1 (smaller), covering the
            # boundary where the PE otherwise catches DVE's xn_b build.
            if s0_chunks and bi == 0:
                s0_direct = s0_chunks
            elif s1_chunks and bi == 1:
                s0_direct = s1_chunks
            else:
                s0_direct = 0
            build_taps = list(s_pe_taps)
            if s0_direct:
                build_taps += [
                    t for t in [*s_dve_taps, s_extra] if t is not None
                ]
            skip_rows = s0_direct * CH

            # Issue the first image-quarter DMA ahead of everything else so
            # the startup chain (q0 land -> placement copy -> first matmul)
            # isn't queued behind k on the shared DGE.  For sample 0 the
            # quarter is further split so the first few rows (all chunk 0
            # needs) land and place quickly.
            q0_raw = None
            q0_splits = None
            k_sb = kpool.tile([C, 9], f32)
            if w_dma:
                # per-tap weights land via one DMA per sample; issued first
                # so sample 0's weights are there when quarter 0 lands.
                w_all = wpool.tile([C, 9, C], f16)
                nc.sync.dma_start(
                    w_all[:], wall_d[b].rearrange("c (t o) -> c t o", o=C)
                )
            if f16_in and not (dma_direct and f16_in):
                q0_raw = xraw_pool.tile([C, QROWS, W], f16)
                q0_splits = [8, QROWS] if bi == 0 else [QROWS]
                rprev = 0
                for qi, rsp in enumerate(q0_splits):
                    nc.sync.dma_start(
                        q0_raw[:, rprev:rsp, :],
                        x_d[b, :, rprev * W : rsp * W].rearrange(
                            "c (h w) -> c h w", w=W
                        ),
                    )
                    rprev = rsp
                    if qi == 0:
                        # k squeezes between the q0 halves: lands early
                        # enough for the w_all weight builds without
                        # delaying the first image rows.
                        nc.sync.dma_start(k_sb[:], k_d[b])
            else:
                nc.sync.dma_start(k_sb[:], k_d[b])

            # Per-tap PE weights: W_t[c, o] = (pw[o,c]*inv[o]) * k[b,c,t]
            if w_dma:
                tap_idx = {
                    (dy, dx): dy * 3 + dx
                    for dy in range(3)
                    for dx in range(3)
                }
            else:
                w_all = wpool.tile([C, len(build_taps), C], f16)
                for i, (dy, dx) in enumerate(build_taps):
                    t = dy * 3 + dx
                    if w_act:
                        # weight builds on ACT (headroom) free DVE cycles
                        nc.scalar.activation(
                            w_all[:, i, :],
                            w_sb[:],
                            mybir.ActivationFunctionType.Copy,
                            bias=0.0,
                            scale=k_sb[:, t : t + 1],
                        )
                    else:
                        nc.vector.tensor_scalar_mul(
                            w_all[:, i, :], w_sb[:], k_sb[:, t : t + 1]
                        )
                tap_idx = {t: i for i, t in enumerate(build_taps)}

            return dict(
                bi=bi, b=b, w_all=w_all, k_sb=k_sb, tap_idx=tap_idx,
                q0_raw=q0_raw, q0_splits=q0_splits,
                s_pe_taps=s_pe_taps, s_dve_taps=s_dve_taps,
                s_extra=s_extra, s_xrows=s_xrows,
                s0_direct=s0_direct, build_taps=build_taps,
                skip_rows=skip_rows,
                last=bi == BPC - 1,
            )

        def emit_prep_rest(pw):
            bi, b = pw["bi"], pw["b"]
            k_sb = pw["k_sb"]
            q0_raw, q0_splits = pw["q0_raw"], pw["q0_splits"]
            s_dve_taps = pw["s_dve_taps"]
            s_extra, s_xrows = pw["s_extra"], pw["s_xrows"]
            skip_rows = pw["skip_rows"]

            # Zero-padded fp16 image; borders re-zeroed each sample since
            # pool slots are recycled.
            x_pad = xpad_pool.tile([C, PH, PW], f16)
            pw["x_pad"] = x_pad
            nc.gpsimd.memset(x_pad[:, 0, :], 0.0)
            nc.gpsimd.memset(x_pad[:, PH - 1, :], 0.0)
            nc.gpsimd.memset(x_pad[:, 1 : PH - 1, 0:2], 0.0)
            nc.gpsimd.memset(x_pad[:, 1 : PH - 1, PW - 2 : PW], 0.0)
            if f16_in and dma_direct:
                nc.sync.dma_start(
                    x_pad[:, 1 : 1 + QROWS, 2 : W + 2],
                    x_d[b, :, 0 : QROWS * W].rearrange("c (h w) -> c h w", w=W),
                )
            else:
                _placement(bi, 0, x_pad, q0_raw, q0_splits)
            for hh in range(1, 4):
                if f16_in and dma_direct:
                    # DMA straight into the padded interior (strided dest);
                    # skips x_raw staging and the DVE placement copies.
                    r0 = 1 + hh * QROWS
                    nc.sync.dma_start(
                        x_pad[:, r0 : r0 + QROWS, 2 : W + 2],
                        x_d[b, :, hh * QROWS * W : (hh + 1) * QROWS * W]
                        .rearrange("c (h w) -> c h w", w=W),
                    )
                    continue
                x_raw = xraw_pool.tile([C, QROWS, W], f16 if f16_in else f32)
                nc.sync.dma_start(
                    x_raw[:],
                    x_d[b, :, hh * QROWS * W : (hh + 1) * QROWS * W].rearrange(
                        "c (h w) -> c h w", w=W
                    ),
                )
                _placement(bi, hh, x_pad, x_raw, None)

            # Partial depthwise on DVE (taps with dx == 1, aligned reads).
            # split_xnb: compute in two row-parts (split at row 44, inside
            # what the first two cast quarters cover) so early PE chunks
            # unblock before the whole sample's partial sum is done.
            xn_b = None
            if s_dve_taps:
                xn_b = xnb_pool.tile([C, H, W], f16)
                if xnb_parts:
                    nper = max(4, (H // xnb_parts + 3) // 4 * 4)
                    bounds = list(range(0, H, nper)) + [H]
                    parts = list(zip(bounds[:-1], bounds[1:]))
                elif split_xnb:
                    parts = [(0, 44), (44, H)]
                else:
                    parts = [(0, H)]
                # one whole-sample scratch per engine; parts write disjoint
                # row ranges so they don't serialize on buffer reuse.
                tmp_s = tmp_pool.tile([C, H, W], f16)
                if act_tap:
                    tmpa_s = tmpa_pool.tile([C, H, W], f16)
                else:
                    tmpa_s = None
                for r0, r1 in parts:
                    if r1 <= skip_rows:
                        continue  # rows covered by sample-0-lite direct taps
                    nr = r1 - r0
                    part_taps = list(s_dve_taps)
                    if s_extra is not None and r0 < s_xrows:
                        part_taps.append(s_extra)  # clipped below
                    (dy0, dx0) = part_taps[0]
                    t0 = dy0 * 3 + dx0
                    nc.vector.tensor_scalar_mul(
                        xn_b[:, r0:r1, :],
                        x_pad[:, r0 + dy0 : r0 + dy0 + nr, dx0 + 1 : dx0 + 1 + W],
                        k_sb[:, t0 : t0 + 1],
                    )
                    for dy, dx in part_taps[1:]:
                        t = dy * 3 + dx
                        is_extra = (dy, dx) == s_extra
                        rr1 = min(r1, s_xrows) if is_extra else r1
                        nrr = rr1 - r0
                        if is_extra and act_tap:
                            tmp = tmpa_s
                            nc.scalar.activation(
                                tmp[:, r0:rr1, :],
                                x_pad[:, r0 + dy : r0 + dy + nrr,
                                      dx + 1 : dx + 1 + W],
                                mybir.ActivationFunctionType.Copy,
                                bias=0.0,
                                scale=k_sb[:, t : t + 1],
                            )
                        else:
                            tmp = tmp_s
                            # act2: tail rows of the last dve tap's product
                            # go to the ScalarEngine (headroom) so DVE gains
                            # slack for next-sample prep.
                            asplit = max(r0, min(rr1, H - act2_rows)) \
                                if ((dy, dx) == s_dve_taps[-1] and act2_rows
                                    and bi > 0) else rr1
                            if asplit > r0:
                                nc.vector.tensor_scalar_mul(
                                    tmp[:, r0:asplit, :],
                                    x_pad[:, r0 + dy : r0 + dy + (asplit - r0),
                                          dx + 1 : dx + 1 + W],
                                    k_sb[:, t : t + 1],
                                )
                            if rr1 > asplit:
                                nc.scalar.activation(
                                    tmp[:, asplit:rr1, :],
                                    x_pad[:, asplit + dy : asplit + dy
                                          + (rr1 - asplit),
                                          dx + 1 : dx + 1 + W],
                                    mybir.ActivationFunctionType.Copy,
                                    bias=0.0,
                                    scale=k_sb[:, t : t + 1],
                                )
                        gsplit = max(r0, min(rr1, H - gps_add_rows)) \
                            if (is_extra and act_tap and gps_add_rows) else rr1
                        if gsplit > r0:
                            nc.vector.tensor_add(
                                xn_b[:, r0:gsplit, :].rearrange("c h w -> c (h w)"),
                                xn_b[:, r0:gsplit, :].rearrange("c h w -> c (h w)"),
                                tmp[:, r0:gsplit, :].rearrange("c h w -> c (h w)"),
                            )
                        if rr1 > gsplit:
                            # act-tap merge for the trailing rows goes to
                            # GpSimd (idle capacity); they are consumed last
                            # by the PE folds, hiding GpSimd's lower rate.
                            nc.gpsimd.tensor_add(
                                xn_b[:, gsplit:rr1, :].rearrange("c h w -> c (h w)"),
                                xn_b[:, gsplit:rr1, :].rearrange("c h w -> c (h w)"),
                                tmp[:, gsplit:rr1, :].rearrange("c h w -> c (h w)"),
                            )
                    # second off-PE tap (EXTRA2): ACT product + DVE merge
                    # for rows [0, extra2_rows).  Skipped for sample-0-lite
                    # rows (covered by direct PE taps).
                    if extra2_rows and r0 < extra2_rows and r1 > skip_rows:
                        e0 = max(r0, skip_rows if bi == 0 else r0)
                        e1 = min(r1, extra2_rows)
                        if e1 > e0:
                            dy, dx = EXTRA2
                            t = dy * 3 + dx
                            if "tmpb_s" not in pw:
                                pw["tmpb_s"] = tmpb_pool.tile(
                                    [C, H, W], f16, name="tmpb_s"
                                )
                            tmpb_s = pw["tmpb_s"]
                            if extra2_act:
                                nc.scalar.activation(
                                    tmpb_s[:, e0:e1, :],
                                    x_pad[:, e0 + dy : e1 + dy,
                                          dx + 1 : dx + 1 + W],
                                    mybir.ActivationFunctionType.Copy,
                                    bias=0.0,
                                    scale=k_sb[:, t : t + 1],
                                )
                            else:
                                # misaligned read (dx+1 odd) -> 2x_2p mode
                                nc.vector.tensor_scalar_mul(
                                    tmpb_s[:, e0:e1, :],
                                    x_pad[:, e0 + dy : e1 + dy,
                                          dx + 1 : dx + 1 + W],
                                    k_sb[:, t : t + 1],
                                )
                            nc.vector.tensor_add(
                                xn_b[:, e0:e1, :].rearrange("c h w -> c (h w)"),
                                xn_b[:, e0:e1, :].rearrange("c h w -> c (h w)"),
                                tmpb_s[:, e0:e1, :].rearrange("c h w -> c (h w)"),
                            )
            return dict(pw, xn_b=xn_b)

        def emit_prep(bi, b):
            return emit_prep_rest(emit_prep_w(bi, b))

        def emit_compute(st):
            b = st["b"]
            x_pad, xn_b, w_all = st["x_pad"], st["xn_b"], st["w_all"]
            s_pe_taps, s_dve_taps = st["s_pe_taps"], st["s_dve_taps"]
            s_extra, s_xrows = st["s_extra"], st["s_xrows"]
            def emit_chunk_mms(ps_slice, h0):
                tap_idx = st["tap_idx"]
                if st["s0_direct"] and h0 + CH <= st["s0_direct"] * CH:
                    taps = st["build_taps"]
                    for i, (dy, dx) in enumerate(taps):
                        nc.tensor.matmul(
                            ps_slice,
                            w_all[:, tap_idx[(dy, dx)], :],
                            x_pad[:, h0 + dy : h0 + dy + CH,
                                  dx + 1 : dx + 1 + W],
                            start=(i == 0),
                            stop=(i == len(taps) - 1),
                        )
                    return
                x2rows = extra2_rows if s_dve_taps else 0
                chunk_pe_taps = [
                    t
                    for t in s_pe_taps
                    if not (t == s_extra and h0 + CH <= s_xrows)
                    and not (t == EXTRA2 and h0 + CH <= x2rows)
                ]
                nmm = len(chunk_pe_taps) + (1 if s_dve_taps else 0)
                mi = 0
                for dy, dx in chunk_pe_taps:
                    nc.tensor.matmul(
                        ps_slice,
                        w_all[:, tap_idx[(dy, dx)], :],
                        x_pad[:, h0 + dy : h0 + dy + CH, dx + 1 : dx + 1 + W],
                        start=(mi == 0),
                        stop=(mi == nmm - 1),
                    )
                    mi += 1
                if s_dve_taps:
                    nc.tensor.matmul(
                        ps_slice,
                        w16[:],
                        xn_b[:, h0 : h0 + CH, :],
                        start=(mi == 0),
                        stop=(mi == nmm - 1),
                    )
                    mi += 1

            for half in range(4):
                o_sb = opool.tile([C, HW // 4], out16 if f16_out else f32)
                if bn_pair:
                    for pi in range(NCHUNK // 8):
                        ps = pspool.tile([C, 2, 512], f32)
                        for j in range(2):
                            h0 = (half * (NCHUNK // 4) + pi * 2 + j) * CH
                            emit_chunk_mms(ps[:, j, 0:NFREE], h0)
                        o_slice = o_sb[
                            :, pi * 2 * NFREE : (pi + 1) * 2 * NFREE
                        ].rearrange("c (j f) -> c j f", j=2)
                        if pi < bn_gps:
                            # BN+ReLU on the (otherwise idle) GpSimd engine:
                            # out = max(psum + beta, 0)
                            nc.gpsimd.tensor_scalar(
                                o_slice,
                                ps[:, :, 0:NFREE],
                                beta_sb[:],
                                0.0,
                                mybir.AluOpType.add,
                                mybir.AluOpType.max,
                            )
                        else:
                            nc.scalar.activation(
                                o_slice,
                                ps[:, :, 0:NFREE],
                                relu,
                                bias=beta_sb[:],
                                scale=1.0,
                            )
                else:
                    for ci in range(NCHUNK // 4):
                        h0 = (half * (NCHUNK // 4) + ci) * CH
                        ps = pspool.tile([C, NFREE], f32)
                        emit_chunk_mms(ps[:], h0)
                        nc.scalar.activation(
                            o_sb[:, ci * NFREE : (ci + 1) * NFREE],
                            ps[:],
                            relu,
                            bias=beta_sb[:],
                            scale=1.0,
                        )
                if st.get("last") and half == 3 and bn_pair:
                    # final quarter streams out per pair-group so the last
                    # DMA is small and the drain tail is short
                    for pi in range(NCHUNK // 8):
                        f0, f1 = pi * 2 * NFREE, (pi + 1) * 2 * NFREE
                        nc.sync.dma_start(
                            out_d[b, :, half * (HW // 4) + f0 : half * (HW // 4) + f1],
                            o_sb[:, f0:f1],
                        )
                else:
                    nc.sync.dma_start(
                        out_d[b, :, half * (HW // 4) : (half + 1) * (HW // 4)],
                        o_sb[:],
                    )

        samples = [b for _ in range(repeat) for b in range(BPC)]
        if hoist:
            # w-phase of sample i+1 is emitted before the xn_b phase of
            # sample i, so next-sample weights/k/q0 sit ahead of the bulky
            # xn_b work in the DVE/SP streams and the PE never waits for
            # them at sample boundaries.
            pw = emit_prep_w(0, samples[0])
            for i in range(len(samples)):
                pw_next = (
                    emit_prep_w(i + 1, samples[i + 1])
                    if i + 1 < len(samples)
                    else None
                )
                emit_compute(emit_prep_rest(pw))
                pw = pw_next
        elif skew:
            # software-pipelined emission: prep(i+1) is emitted before
            # compute(i) so next-sample casts/products aren't queued behind
            # the current sample's 24 BN ops on ACT/DVE.
            pend = emit_prep(0, samples[0])
            for i in range(len(samples)):
                nxt = emit_prep(i + 1, samples[i + 1]) if i + 1 < len(samples) else None
                emit_compute(pend)
                pend = nxt
        else:
            for bi, b in enumerate(samples):
                emit_compute(emit_prep(bi, b))

    nc.compile()
    return nc


def prep_in_maps(x, k, pw_weight, bn_gamma, bn_beta, bn_mean, bn_var,
                 build_kw=None):
    """Host-side input prep + per-core sharding (data-parallel over batch)."""
    build_kw = BEST_KW if build_kw is None else build_kw
    # 16-bit on host: bit-identical to the on-device cast the kernel used to
    # do, but halves the input DMA bytes.
    if build_kw.get("bf16"):
        import ml_dtypes

        x16 = ml_dtypes.bfloat16
    else:
        x16 = np.float16
    x = np.ascontiguousarray(
        np.asarray(x, dtype=np.float32).reshape(B, C, HW).astype(x16)
    )
    k = np.ascontiguousarray(np.asarray(k, dtype=np.float32)).reshape(B, C, 9)
    pw_weight = np.asarray(pw_weight, dtype=np.float32)
    inv = np.asarray(bn_gamma, np.float32) / np.sqrt(
        np.asarray(bn_var, np.float32) + BN_EPS
    )
    # lhsT layout [c, o] with BN scale folded in.
    w_dt = x16 if build_kw.get("f16_w") else np.float32
    w_eff32 = (pw_weight * inv[:, None]).T.astype(np.float32)
    w_eff = np.ascontiguousarray(w_eff32.astype(w_dt))
    beta_eff = np.ascontiguousarray(
        (np.asarray(bn_beta, np.float32) - np.asarray(bn_mean, np.float32) * inv)
        .astype(np.float32)
        .reshape(C, 1)
    )
    wall = None
    if build_kw.get("w_dma"):
        # per-sample per-tap PE weights, precomputed on host:
        # wall[b, c, t, o] = w_eff[c, o] * k[b, c, t]
        wall = np.ascontiguousarray(
            (k[:, :, :, None] * w_eff32[:, None, :])
            .astype(np.float16)
            .reshape(B, C, 9 * C)
        )
    return [
        {
            "x": x[c * BPC : (c + 1) * BPC],
            "k": k[c * BPC : (c + 1) * BPC],
            "w": w_eff,
            "beta": beta_eff,
            **(
                {"wall": wall[c * BPC : (c + 1) * BPC]}
                if wall is not None
                else {}
            ),
        }
        for c in range(NCORES)
    ]


def kernel(x, k, pw_weight, bn_gamma, bn_beta, bn_mean, bn_var):
    global _compiled
    from concourse.bass_utils import run_bass_kernel_spmd

    in_maps = prep_in_maps(
        x, k, pw_weight, bn_gamma, bn_beta, bn_mean, bn_var
    )
    if _compiled is None:
        _compiled = _build(**BEST_KW)
    nc = _compiled
    trace = bool(int(os.environ.get("KERNEL_TRACE", "0")))
    try:
        res = run_bass_kernel_spmd(
            nc, in_maps, core_ids=list(range(NCORES)), trace=trace
        )
    except ModuleNotFoundError:
        # NTFF trace hook unavailable under this axon client; run untraced.
        trace = False
        res = run_bass_kernel_spmd(
            nc, in_maps, core_ids=list(range(NCORES)), trace=False
        )
    if trace and res.exec_time_ns is not None:
        print(f"HW exec time: {res.exec_time_ns} ns")
        kernel.last_exec_time_ns = res.exec_time_ns
        kernel.last_trace = res.instructions_and_trace
    out = np.concatenate([r["out"] for r in res.results], axis=0)
    return out.reshape(B, C, H, W).astype(np.float32, copy=False)

